# revision 17
# baseline (speedup 1.0000x reference)
"""BiLSTM tagger on 8 TRN2 NeuronCores.

Strategy (hardcoded for B=64,T=512,V=30000,E=128,H=256,TAGS=50):
  - Data-parallel: batch sharded 8 ways (8 sequences/core); weights replicated.
  - Per core: embedding gather (indirect DMA) -> PE transpose -> x^T in SBUF;
    input projections xg = W_ih_aug @ [x; 1-m; 1] precomputed for all t as big
    matmuls into DRAM scratch (middle-out tile order so ph1 can start after
    2 tiles); recurrences fully STATICALLY UNROLLED (no dynamic loops), xb
    DMAs prefetched one 64-step body ahead; classifier.
  - ALL-SIGMOID formulation: tanh(x) = 2*sigmoid(2x)-1 folded into weight
    scaling. Carried state is h/2, c/2; consumers' weights pre-scaled by 2
    (Whh, l2 Wih, cls_W), g-gate rows by an extra 2. Every activation is
    Sigmoid (tanh(c) = sig(4*(c/2)) via the activation scale arg) -> zero
    act-table switches.
  - Per unit-step: DVE prefills PSUM with xg (h-independent, off critical
    path); 12 Whh matmuls for i,f,g chunks then sig(ifg) fires while the 4
    o-chunk matmuls still run; DVE u1=(sg-.5)*si, csf=sf*c', c'=csf+u1;
    Act sig(4c'); DVE h'=(sc-.5)*so -> staged bf16.
  - Masking: +/-60*(1-m) on i/f gate pre-activations freezes c exactly at
    masked steps; backward h is exactly 0 there. The l2f held output is
    reconstructed per-body with tensor_tensor_scan along t:
    held[t] = (1-m[t])*held[t-1] + m[t]*h[t].
"""
import sys

sys.path.insert(0, "/opt/trn_rl_repo")
import contextlib

import numpy as np
import ml_dtypes

import concourse.bass as bass
import concourse.bacc as bacc
import concourse.mybir as mybir
import concourse.tile as tile
from concourse.bass_utils import run_bass_kernel_spmd
from concourse.masks import make_identity

B, T, V, E, H, TAGS = 64, 512, 30000, 128, 256, 50
NCORES = 8
Bc = B // NCORES          # 8 sequences per core
TB = T * Bc               # 4096 tokens per core
SPB = 64                  # steps per body
NBODY = T // SPB          # 8

f32 = mybir.dt.float32
bf16 = mybir.dt.bfloat16
i32 = mybir.dt.int32

UNITS = ("1f", "1b", "2f", "2b")
KCNT = {"1f": 1, "1b": 1, "2f": 4, "2b": 4}       # 128-row K chunks of x features
MCNT = {"1f": 8, "1b": 8, "2f": 10, "2b": 8}      # 128-row output chunks
REV = {"1f": False, "1b": True, "2f": False, "2b": True}

_CACHE = {}

SIG = mybir.ActivationFunctionType.Sigmoid
ADD = mybir.AluOpType.add
MUL = mybir.AluOpType.mult

# gate-row order [i(0:256), f(256:512), g(512:768), o(768:1024)] = torch order
# with g and o swapped
PERM = np.concatenate([np.arange(0, 512), np.arange(512, 768),
                       np.arange(768, 1024)])  # identity on i,f; then g; then o


def _prep_unit_weights(Wih, Whh, bih, bhh, m_cnt, in_scale):
    """Host-side weight marshalling (all-sigmoid form).

    Torch row order is [i f g o]; we keep it (i=chunks0:2, f=2:4, g=4:6,
    o=6:8). in_scale compensates h/2-scaled inputs (2.0 for l2). Whh x2
    (recurrent h is h/2); g rows an extra x2 (tanh = 2*sig(2x)-1)."""
    din = Wih.shape[1]
    Wp = np.asarray(Wih, np.float64) * in_scale   # [1024, din]
    Up = np.asarray(Whh, np.float64) * 2.0        # [1024, 256]
    bp = (np.asarray(bih, np.float64) + np.asarray(bhh, np.float64)).copy()
    Wp = Wp.copy()
    Wp[512:768] *= 2.0
    Up = Up.copy()
    Up[512:768] *= 2.0
    bp[512:768] *= 2.0
    M = m_cnt * 128
    k_cnt = din // 128
    # x-part lhsT: [din, M] -> k-chunk-major cols [128, k_cnt*M]
    WT = np.zeros((din, M), np.float64)
    WT[:, :1024] = Wp.T
    wx = np.concatenate([WT[k * 128:(k + 1) * 128, :] for k in range(k_cnt)],
                        axis=1).astype(ml_dtypes.bfloat16)  # [128, k_cnt*M]
    # aug lhsT rows: feature0 = (1-m), feature1 = 1
    wa = np.zeros((2, M), np.float64)
    wa[0, 0:256] = -60.0   # i rows: -60*(1-m)
    wa[0, 256:512] = 60.0  # f rows: +60*(1-m)
    wa[1, :1024] = bp
    if m_cnt == 10:        # l2f extra planes: chunk8 = m, chunk9 = 1-m
        wa[0, 1024:1152] = -1.0
        wa[1, 1024:1152] = 1.0
        wa[0, 1152:1280] = 1.0
        wa[1, 1152:1280] = 0.0
    wa = wa.astype(ml_dtypes.bfloat16)
    # Whh lhsT: [256, 1024] -> [128, 2*1024], (k*8+m) chunk indexing
    UT = Up.T
    wh = np.concatenate([UT[0:128, :], UT[128:256, :]], axis=1).astype(ml_dtypes.bfloat16)
    return wx, wa, wh


def _build_program():
    nc = bacc.Bacc("TRN2", target_bir_lowering=False, debug=False, num_devices=NCORES)
    emb_d = nc.dram_tensor("emb", [V, E], f32, kind="ExternalInput")
    words_d = nc.dram_tensor("words", [TB, 1], i32, kind="ExternalInput")
    aug_d = nc.dram_tensor("aug", [2, TB], bf16, kind="ExternalInput")
    wxd, wad, whd, xgd = {}, {}, {}, {}
    for u in UNITS:
        wxd[u] = nc.dram_tensor(f"w{u}x", [128, KCNT[u] * MCNT[u] * 128], bf16, kind="ExternalInput")
        wad[u] = nc.dram_tensor(f"w{u}a", [2, MCNT[u] * 128], bf16, kind="ExternalInput")
        whd[u] = nc.dram_tensor(f"w{u}h", [128, 2048], bf16, kind="ExternalInput")
        xgd[u] = [nc.dram_tensor(f"xg{u}t{t}", [128, MCNT[u], SPB, Bc], bf16)
                  for t in range(NBODY)]
    clsx_d = nc.dram_tensor("clsx", [128, 4 * TAGS], bf16, kind="ExternalInput")
    clsb_d = nc.dram_tensor("clsb", [TAGS, 1], f32, kind="ExternalInput")
    logits_d = nc.dram_tensor("logits", [TAGS, TB], f32, kind="ExternalOutput")

    ctx = contextlib.ExitStack()
    with tile.TileContext(nc) as tc, ctx:
        pp = ctx.enter_context(tc.tile_pool(name="persist", bufs=1))
        aug_sb = pp.tile([2, TB], bf16, tag="aug")
        wh_sb = {u: pp.tile([128, 2048], bf16, tag=f"wh{u}", name=f"wh{u}") for u in UNITS}
        cls_sb = pp.tile([128, 4 * TAGS], bf16, tag="clsx")
        clsb_sb = pp.tile([TAGS, 1], f32, tag="clsb")
        hs = {u: pp.tile([128, T, 2, Bc], bf16, tag=f"hs{u}", name=f"hs{u}")
              for u in ("1f", "1b", "2b")}
        o2f = pp.tile([128, T + 1, 2, Bc], bf16, tag="o2f")  # col0 = zeros
        identb = pp.tile([128, 128], bf16, tag="identb")
        hcar = {u: pp.tile([128, 2, Bc], bf16, tag=f"hc{u}", name=f"hc{u}") for u in UNITS}
        ccar = {u: pp.tile([128, 2, Bc], f32, tag=f"cc{u}", name=f"cc{u}") for u in UNITS}

        for u in UNITS:
            nc.sync.dma_start(wh_sb[u][:], whd[u][:])
        nc.sync.dma_start(cls_sb[:], clsx_d[:])
        nc.sync.dma_start(clsb_sb[:], clsb_d[:])
        nc.sync.dma_start(aug_sb[:], aug_d[:])
        make_identity(nc, identb[:])
        for u in UNITS:
            nc.vector.memset(hcar[u][:, :, :], 0.0)
            nc.vector.memset(ccar[u][:, :, :], 0.0)
        nc.vector.memset(o2f[:, 0, :, :], 0.0)

        # ---- prologue: gather + xg1 interleaved, middle-out tile order
        with nc.named_scope("gather"), \
             tc.tile_pool(name="prolog", bufs=1) as lp, \
             tc.tile_pool(name="gat", bufs=4) as gp, \
             tc.tile_pool(name="gps", bufs=2, space="PSUM") as gps, \
             tc.tile_pool(name="xp1", bufs=2, space="PSUM") as xps, \
             tc.tile_pool(name="xs1", bufs=2) as xsb:
            xT = lp.tile([128, TB], bf16, tag="xT")
            ident = lp.tile([128, 128], f32, tag="ident")
            make_identity(nc, ident[:])
            wx1, wa1 = {}, {}
            for u in ("1f", "1b"):
                wx1[u] = lp.tile([128, KCNT[u] * MCNT[u] * 128], bf16,
                                 tag=f"wx{u}", name=f"wx{u}")
                nc.sync.dma_start(wx1[u][:], wxd[u][:])
                wa1[u] = lp.tile([2, MCNT[u] * 128], bf16, tag=f"wa{u}",
                                 name=f"wa{u}")
                nc.sync.dma_start(wa1[u][:], wad[u][:])

            def l1_rhs(k, n):
                return xT[:, n * 512:(n + 1) * 512]

            def gather_chunk(n):
                def go():
                    idx = gp.tile([128, 1], i32, tag="idx")
                    nc.sync.dma_start(idx[:], words_d[n * 128:(n + 1) * 128, :])
                    xt = gp.tile([128, 128], f32, tag="xt")
                    nc.gpsimd.indirect_dma_start(
                        out=xt[:], out_offset=None, in_=emb_d[:, :],
                        in_offset=bass.IndirectOffsetOnAxis(ap=idx[:, :1], axis=0))
                    pst = gps.tile([128, 128], f32, tag="pst")
                    nc.tensor.transpose(out=pst[:], in_=xt[:], identity=ident[:])
                    nc.vector.tensor_copy(xT[:, n * 128:(n + 1) * 128], pst[:])
                return go

            def xg1_pair_closures(p):
                # pair p = tiles {p, 7-p}; ph1 body i consumes 1f tile i and
                # 1b tile 7-i, i.e. pair min(i, 7-i).
                fns = []
                for tI in (p, 7 - p):
                    fns += [gather_chunk(tI * 4 + j) for j in range(4)]
                    for u in ("1f", "1b"):
                        fns += xg_tile_closures(nc, u, tI, l1_rhs, wx1[u],
                                                wa1[u], aug_sb, xgd[u],
                                                xps, xsb)
                return fns

            for fn in xg1_pair_closures(0):
                fn()

            def pre_body1(i):
                if i + 1 <= 3:
                    return xg1_pair_closures(i + 1)
                return []

            phase(nc, tc, ("1f", "1b"), wh_sb, hcar, ccar, hs, o2f, xgd,
                  identb, pre_body=pre_body1)

        # ---- xg2 (consumes hs1f/hs1b)
        def l2_rhs(k, n):
            src = hs["1f"] if k < 2 else hs["1b"]
            return src[:, n * 64:(n + 1) * 64, k % 2, :]

        with nc.named_scope("xg2"), \
             tc.tile_pool(name="xg2w", bufs=1) as xwp, \
             tc.tile_pool(name="xp2", bufs=4, space="PSUM") as xps, \
             tc.tile_pool(name="xs2", bufs=2) as xsb:
            wx2, wa2 = {}, {}
            for u in ("2f", "2b"):
                wx2[u] = xwp.tile([128, KCNT[u] * MCNT[u] * 128], bf16,
                                  tag=f"wx{u}", name=f"wx2{u}")
                nc.sync.dma_start(wx2[u][:], wxd[u][:])
                wa2[u] = xwp.tile([2, MCNT[u] * 128], bf16, tag=f"wa{u}",
                                  name=f"wa2{u}")
                nc.sync.dma_start(wa2[u][:], wad[u][:])

            def xg2_pair_closures(p):
                # pair p = {2f tile p, 2b tile 7-p}; ph2 body i consumes
                # 2f tile i and 2b tile 7-i, i.e. exactly pair i.
                return (xg_tile_closures(nc, "2f", p, l2_rhs, wx2["2f"],
                                         wa2["2f"], aug_sb, xgd["2f"],
                                         xps, xsb)
                        + xg_tile_closures(nc, "2b", 7 - p, l2_rhs, wx2["2b"],
                                           wa2["2b"], aug_sb, xgd["2b"],
                                           xps, xsb))

            for fn in xg2_pair_closures(0):
                fn()

            def pre_body2(i):
                if i + 1 < NBODY:
                    return xg2_pair_closures(i + 1)
                return []

            phase(nc, tc, ("2f", "2b"), wh_sb, hcar, ccar, hs, o2f, xgd,
                  identb, pre_body=pre_body2)

        # ---- classifier
        with nc.named_scope("cls"), \
             tc.tile_pool(name="cl", bufs=3) as cp, \
             tc.tile_pool(name="cps", bufs=3, space="PSUM") as cps:
            for n in range(TB // 512):
                psm = cps.tile([TAGS, 512], f32, tag="ps")
                for k in range(4):
                    if k < 2:
                        rhs = o2f[:, 1 + n * 64:1 + (n + 1) * 64, k, :]
                    else:
                        rhs = hs["2b"][:, n * 64:(n + 1) * 64, k % 2, :]
                    nc.tensor.matmul(
                        out=psm[:],
                        lhsT=cls_sb[:, k * TAGS:(k + 1) * TAGS],
                        rhs=rhs,
                        start=(k == 0), stop=(k == 3))
                lg = cp.tile([TAGS, 512], f32, tag="lg")
                nc.vector.tensor_scalar_add(lg[:], psm[:], clsb_sb[:, :1])
                nc.sync.dma_start(logits_d[:, n * 512:(n + 1) * 512], lg[:])

    nc.compile()
    return nc


def xg_tile_closures(nc, u, tI, rhs_of_k, wx_sb, wa_sb, aug_sb, xg_dram,
                     xps, xsb):
    """Closures for one 512-token xg tile: one per m-chunk + final DMA."""
    m_cnt, k_cnt = MCNT[u], KCNT[u]
    n = tI
    nsl = slice(n * 512, (n + 1) * 512)
    box = {}

    def mk_alloc():
        box["stg"] = xsb.tile([128, m_cnt, 512], bf16, tag=f"stg{u}",
                              name=f"stg{u}")

    def mk_chunk(m):
        def go():
            psm = xps.tile([128, 512], f32, tag="ps")
            first = True
            if m < 8:  # gate chunks get the x contribution
                for k in range(k_cnt):
                    nc.tensor.matmul(
                        out=psm[:],
                        lhsT=wx_sb[:, (k * m_cnt + m) * 128:(k * m_cnt + m + 1) * 128],
                        rhs=rhs_of_k(k, n),
                        start=first, stop=False)
                    first = False
            nc.tensor.matmul(
                out=psm[:],
                lhsT=wa_sb[:, m * 128:(m + 1) * 128],
                rhs=aug_sb[:, nsl],
                start=first, stop=True)
            stg = box["stg"]
            if (n + m) % 2 == 0:
                nc.vector.tensor_copy(stg[:, m, :], psm[:])
            else:
                nc.scalar.activation(stg[:, m, :], psm[:],
                                     mybir.ActivationFunctionType.Copy)
        return go

    def mk_dma():
        nc.sync.dma_start(
            xg_dram[tI][:, :, :, :],
            box["stg"][:, :, :].rearrange("p m (t b) -> p m t b", b=Bc))

    return [mk_alloc] + [mk_chunk(m) for m in range(m_cnt)] + [mk_dma]


def xg_tile(nc, u, tI, rhs_of_k, wx_sb, wa_sb, aug_sb, xg_dram, xps, xsb):
    for fn in xg_tile_closures(nc, u, tI, rhs_of_k, wx_sb, wa_sb, aug_sb,
                               xg_dram, xps, xsb):
        fn()


def phase(nc, tc, units, wh_sb, hcar, ccar, hs, o2f, xgd, identb,
          pre_body=None):
    """Static-unrolled recurrence for two direction units.

    Unit B is emitted ONE FULL STEP behind unit A: in the in-order PE stream
    A-sweep(t+1) precedes B-sweep(t), so each unit's pointwise tail overlaps
    the other unit's sweep instead of locksteping (tail-block + sweep-block).
    """
    MC = {u: 10 if u == "2f" else 8 for u in units}
    with nc.named_scope(f"ph{units[0]}"), \
         tc.tile_pool(name=f"rc{units[0]}", bufs=2) as rp, \
         tc.tile_pool(name=f"rps{units[0]}", bufs=2, space="PSUM") as rps, \
         tc.tile_pool(name=f"rtmp{units[0]}", bufs=4) as tp, \
         tc.tile_pool(name=f"rsc{units[0]}", bufs=2) as scp:

        def body_t0(u, i):
            return (NBODY - 1 - i) * SPB if REV[u] else i * SPB

        def dma_xb(u, i):
            xbt = rp.tile([128, MC[u], SPB, Bc], bf16, tag=f"xb{u}",
                          name=f"xb{u}")
            tI = (NBODY - 1 - i) if REV[u] else i
            nc.sync.dma_start(xbt[:, :, :, :], xgd[u][tI][:, :, :, :])
            return xbt

        def prefill(psm, u, xbt, slot):
            nc.tensor.matmul(out=psm[:, :, :], lhsT=identb[:],
                             rhs=xbt[:, 0:8, slot, :],
                             start=True, stop=False, skip_group_check=True)

        xb = {u: dma_xb(u, 0) for u in units}
        xb_next = {}
        psum = {}
        for u in units:
            slot0 = (SPB - 1) if REV[u] else 0
            ps = rps.tile([128, 8, Bc], f32, tag=f"ps{u}", name=f"ps{u}")
            prefill(ps, u, xb[u], slot0)
            psum[u] = ps

        hstage = {}
        hstage_prev = {}
        deferred = []

        def emit_step(u, ugs):
            i, us = divmod(ugs, SPB)
            rev = REV[u]
            slot = (SPB - 1 - us) if rev else us
            if us == 0:
                if u == units[0] and pre_body is not None:
                    deferred.extend(pre_body(i))
                hstage[u] = rp.tile([128, SPB, 2, Bc], bf16, tag=f"hst{u}",
                                    name=f"hst{u}")
            if us == 0:
                if i == 0:
                    hprev = hcar[u]
                else:
                    pl = 0 if rev else SPB - 1
                    hprev = hstage_prev[u][:, pl, :, :]
            else:
                pslot = slot + 1 if rev else slot - 1
                hprev = hstage[u][:, pslot, :, :]
            psm = psum[u]
            # k-outer: the first 6 MMs need only h2's k0 half (written first);
            # sig(ifg) fires after the 12 i,f,g MMs while o-chunk MMs run
            for mg in (range(6), range(6, 8)):
                for k in range(2):
                    for m in mg:
                        nc.tensor.matmul(
                            out=psm[:, m, :],
                            lhsT=wh_sb[u][:, (k * 8 + m) * 128:(k * 8 + m + 1) * 128],
                            rhs=hprev[:, k, :],
                            start=False, stop=(k == 1),
                            skip_group_check=True)
            # per-unit prefetch of the next body's xb (after the drain window
            # so deferred xg writes stay ordered before this read)
            if us == 44 and i + 1 < NBODY:
                xb_next[u] = dma_xb(u, i + 1)
            # prefill the NEXT step's PSUM (h-independent)
            nxb = None
            if us + 1 < SPB:
                nslot = (slot - 1) if rev else (slot + 1)
                nxb = xb[u]
            elif i + 1 < NBODY:
                nslot = (SPB - 1) if rev else 0
                nxb = xb_next[u]
            if nxb is not None:
                ps = rps.tile([128, 8, Bc], f32, tag=f"ps{u}", name=f"ps{u}")
                prefill(ps, u, nxb, nslot)
                psum[u] = ps
            # pointwise tail (all-sigmoid)
            sg = tp.tile([128, 8, Bc], f32, tag=f"sg{u}", name=f"sg{u}")
            nc.scalar.activation(sg[:, 0:6, :], psm[:, 0:6, :], SIG)
            nc.scalar.activation(sg[:, 6:8, :], psm[:, 6:8, :], SIG)
            u1 = tp.tile([128, 2, Bc], f32, tag=f"u1{u}", name=f"u1{u}")
            nc.vector.scalar_tensor_tensor(
                out=u1[:, :, :], in0=sg[:, 4:6, :], scalar=-0.5,
                in1=sg[:, 0:2, :], op0=ADD, op1=MUL)
            csf = tp.tile([128, 2, Bc], f32, tag=f"csf{u}", name=f"csf{u}")
            nc.vector.tensor_tensor(out=csf[:, :, :], in0=sg[:, 2:4, :],
                                    in1=ccar[u][:, :, :], op=MUL)
            nc.vector.tensor_tensor(out=ccar[u][:, :, :],
                                    in0=csf[:, :, :], in1=u1[:, :, :], op=ADD)
            sc = tp.tile([128, 2, Bc], f32, tag=f"sc{u}", name=f"sc{u}")
            nc.scalar.activation(sc[:, :, :], ccar[u][:, :, :], SIG, scale=4.0)
            nc.vector.scalar_tensor_tensor(
                out=hstage[u][:, slot, 0, :], in0=sc[:, 0, :],
                scalar=-0.5, in1=sg[:, 6, :], op0=ADD, op1=MUL)
            nc.vector.scalar_tensor_tensor(
                out=hstage[u][:, slot, 1, :], in0=sc[:, 1, :],
                scalar=-0.5, in1=sg[:, 7, :], op0=ADD, op1=MUL)
            if u == units[0] and us < 44:
                budget = 2 if len(deferred) > (44 - us) else 1
                for _ in range(min(budget, len(deferred))):
                    deferred.pop(0)()
            if us == SPB - 1:
                end_body(u, i)

        def end_body(u, i):
            t0 = body_t0(u, i)
            if u != "2f":
                nc.gpsimd.tensor_copy(hs[u][:, t0:t0 + SPB, :, :],
                                      hstage[u][:, :, :, :])
            else:
                # held[t] = (1-m)[t]*held[t-1] + m[t]*h[t] along t.
                # Planes snapshotted (GpSimd) to decouple from the xb ring;
                # scans DEFERRED and spread over later step slots.
                mpl = scp.tile([128, 2, SPB, Bc], bf16, tag="mpl", name="mpl")
                nc.gpsimd.tensor_copy(mpl[:, :, :, :], xb[u][:, 8:10, :, :])
                hst2f = hstage[u]
                for k in range(2):
                    tmp = scp.tile([128, SPB, Bc], bf16, tag=f"tmp{k}",
                                   name=f"tmp{k}")
                    nc.gpsimd.tensor_tensor(out=tmp[:, :, :],
                                            in0=hst2f[:, :, k, :],
                                            in1=mpl[:, 0, :, :], op=MUL)
                    for b in range(Bc):
                        def mk_scan(k=k, b=b, t0=t0, tmp=tmp, mpl=mpl):
                            nc.vector.tensor_tensor_scan(
                                out=o2f[:, 1 + t0:1 + t0 + SPB, k, b],
                                data0=mpl[:, 1, :, b],
                                data1=tmp[:, :, b],
                                initial=o2f[:, t0, k, b:b + 1],
                                op0=MUL, op1=ADD)
                        deferred.append(mk_scan)
            if i + 1 < NBODY:
                xb[u] = xb_next[u]
            hstage_prev[u] = hstage[u]

        # unit B trails unit A by exactly one step
        for gs in range(T + 1):
            if gs < T:
                emit_step(units[0], gs)
            if gs >= 1:
                emit_step(units[1], gs - 1)
        for fn in deferred:
            fn()


def _make_in_maps(inputs):
    words = np.asarray(inputs["words"]).astype(np.int32)
    lengths = np.asarray(inputs["lengths"]).astype(np.int32)
    emb = np.asarray(inputs["emb"], dtype=np.float32)
    mask = (lengths[:, None] > np.arange(T)[None, :]).astype(np.float32)
    wprep = {u: _prep_unit_weights(inputs[f"l{u}_Wih"], inputs[f"l{u}_Whh"],
                                   inputs[f"l{u}_bih"], inputs[f"l{u}_bhh"],
                                   MCNT[u], 2.0 if u[0] == "2" else 1.0)
             for u in UNITS}
    clsW = np.asarray(inputs["cls_W"], np.float64) * 2.0
    CT = clsW.T
    clsx = np.concatenate([CT[k * 128:(k + 1) * 128, :] for k in range(4)],
                          axis=1).astype(ml_dtypes.bfloat16)
    clsb = np.asarray(inputs["cls_b"], dtype=np.float32).reshape(TAGS, 1)
    in_maps = []
    for c in range(NCORES):
        bsl = slice(c * Bc, (c + 1) * Bc)
        w_c = words[bsl]
        m_c = mask[bsl]
        words_tm = np.ascontiguousarray(w_c.T).reshape(TB, 1)
        aug = np.stack([(1.0 - m_c.T).reshape(TB), np.ones(TB, np.float32)]
                       ).astype(ml_dtypes.bfloat16)
        im = {"emb": emb, "words": words_tm, "aug": aug,
              "clsx": clsx, "clsb": clsb}
        for u in UNITS:
            wx, wa, wh = wprep[u]
            im[f"w{u}x"] = wx
            im[f"w{u}a"] = wa
            im[f"w{u}h"] = wh
        in_maps.append(im)
    return in_maps


def kernel(**inputs):
    if "nc" not in _CACHE:
        _CACHE["nc"] = _build_program()
    nc = _CACHE["nc"]
    in_maps = _make_in_maps(inputs)
    _CACHE["in_maps"] = in_maps
    res = run_bass_kernel_spmd(nc, in_maps, list(range(NCORES)))
    out = np.empty((B, T, TAGS), np.float32)
    for c in range(NCORES):
        lg = res.results[c]["logits"]          # [50, TB], col = t*Bc + b
        out[c * Bc:(c + 1) * Bc] = lg.reshape(TAGS, T, Bc).transpose(2, 1, 0)
    return out


def bench(inputs):
    """Run once with NTFF tracing; returns HW exec_time_ns (and stashes trace)."""
    kernel(**inputs)  # ensure program built/cached
    nc = _CACHE["nc"]
    in_maps = _CACHE["in_maps"]
    import tempfile
    tmpdir = tempfile.mkdtemp(prefix="bilstm_trace_")
    res = run_bass_kernel_spmd(nc, in_maps, list(range(NCORES)), trace=True,
                               tmpdir=tmpdir)
    _CACHE["trace_dir"] = tmpdir
    _CACHE["last_bench"] = res
    print("trace dir:", tmpdir)
    if res.per_core_scope_times:
        for scope, times in res.per_core_scope_times.items():
            print(f"scope {scope}: {times}")
    return res.exec_time_ns


if __name__ == "__main__":
    import reference
    inputs = {k: np.asarray(v) for k, v in reference.setup_inputs().items()}
    got = kernel(**inputs)
    print(got.shape, got.dtype)
